# revision 1
# baseline (speedup 1.0000x reference)
"""Trainium2 Bass kernel for nn_ChamferLoss (reflection-symmetry chamfer loss).

Sharding: pure data parallel - batch b -> core b (B=8, 8 cores). Each core
computes its batch's bidirectional chamfer sums over 3 reflection heads plus
the orthogonality regularizer; the host sums the 8 scalar partials.

Distance matmul: d[i,j] = |x_i|^2 + |y_j|^2 - 2 x_i.y_j on the PE with fp32
operands decomposed into 3 bf16 levels (hh,hm,hl,mh,mm,lh cross terms)
stacked along K=24 -> full-speed bf16 matmul at fp32-grade accuracy (~1e-6
abs on d). fp32 matmul itself runs at 1/4 rate, hence the split trick.
Both chamfer directions are computed with their own matmuls (d and d^T) so
every min is a free-dim reduction. Reflected points never touch DRAM: the
reflection, |y|^2 (= |x|^2 + 4*s*off), the bf16 splits, and the K-stacked
operand tensors are all built on-chip in a [128, rows, tiles] layout and
PE-transposed into matmul form.

Min consumption (tensor_tensor_reduce hangs this runtime's DVE, so it is
not used): per 128-row block, 4 PSUM stripes of 1024 distances are reduced
by a mix of two flavors - fp16-tree blocks (ACT evacuates stripes to SBUF
fp16, DVE runs a 2x-mode TT-min tree; fp16 rounding only perturbs selected
minima by ~d*2^-11) and fp32-chain blocks (DVE TT-mins straight from PSUM,
no ACT traffic). A 3/7 evenly-spread chain fraction balances ACT vs DVE;
cost-model timeline: ~721 us/core, DVE 93% busy.
"""

import sys

sys.path.insert(0, "/opt/trn_rl_repo")

from contextlib import ExitStack

import numpy as np

import concourse.bass as bass
import concourse.bacc as bacc
import concourse.tile as tile
from concourse import mybir
from concourse.masks import make_identity
from concourse.bass_utils import run_bass_kernel_spmd

F32 = mybir.dt.float32
BF16 = mybir.dt.bfloat16
FP16 = mybir.dt.float16
AX = mybir.AxisListType
OP = mybir.AluOpType
AF = mybir.ActivationFunctionType

P = 128
H = 3
REG_COEF = 25.0
B = 8

# level patterns for the 6 kept cross products (x-level, y-level):
# (h,h) (h,m) (h,l) (m,h) (m,m) (l,h)
L_LEVELS = [0, 0, 0, 1, 1, 2]  # stationary-side level per 3-row group
R_LEVELS = [0, 1, 2, 0, 1, 0]  # moving-side level per 3-row group


import os
ASM_ENGINE = os.environ.get("CHAMFER_ASM_ENGINE", "gpsimd")


def ASM(nc):
    return getattr(nc, ASM_ENGINE)


def _split3(nc, pool, src, shape, tag):
    """3-level bf16 split of an f32 tile: src ~= b0+b1+b2 (rel ~2^-25)."""
    outs = []
    cur = src
    for lv in range(3):
        b = pool.tile(shape, BF16, tag=f"{tag}b{lv}")
        nc.scalar.copy(out=b, in_=cur)
        outs.append(b)
        if lv < 2:
            r = pool.tile(shape, F32, tag=f"{tag}r{lv}")
            nc.vector.tensor_tensor(out=r, in0=cur, in1=b, op=OP.subtract)
            cur = r
    return outs


def emit_chamfer(nc, n=4096):
    NT = n // P           # number of 128-point blocks
    W = min(1024, n)      # psum stripe width (free dim)
    NST = n // W          # stripes per row-block
    NMM = W // 512        # matmuls per stripe

    pts = nc.dram_tensor("pts", [n, 3], F32, kind="ExternalInput").ap()
    yp = nc.dram_tensor("yp", [H, 4], F32, kind="ExternalInput").ap()
    out = nc.dram_tensor("out", [1, 1], F32, kind="ExternalOutput").ap()

    with ExitStack() as ctx:
        tc = ctx.enter_context(tile.TileContext(nc))
        const = ctx.enter_context(tc.tile_pool(name="const", bufs=1))
        work = ctx.enter_context(tc.tile_pool(name="work", bufs=4))
        headp = ctx.enter_context(tc.tile_pool(name="headp", bufs=2))
        pstripe = ctx.enter_context(tc.tile_pool(
            name="pstripe", bufs=4, space="PSUM"))

        id128 = const.tile([P, P], BF16)
        make_identity(nc, id128)

        # ---- load points: Xn[p, t, c] = pts[t*128+p, c]
        Xn = const.tile([P, NT, 3], F32)
        nc.sync.dma_start(out=Xn, in_=pts.rearrange("(t p) c -> p t c", p=P))

        # ---- yp broadcast to all partitions: ypb[p, h, k] = yp[h, k]
        ypb = const.tile([P, H, 4], F32)
        yp_b = bass.AP(tensor=yp.tensor, offset=yp.offset,
                       ap=[[0, P], [4, H], [1, 4]])
        nc.sync.dma_start(out=ypb, in_=yp_b)

        # ---- sx = |x|^2 per point, in [128, NT] layout
        Xsq = work.tile([P, NT, 3], F32)
        nc.scalar.activation(out=Xsq, in_=Xn, func=AF.Square)
        sx = const.tile([P, NT], F32)
        nc.vector.tensor_tensor(out=sx, in0=Xsq[:, :, 0], in1=Xsq[:, :, 1], op=OP.add)
        nc.vector.tensor_tensor(out=sx, in0=sx, in1=Xsq[:, :, 2], op=OP.add)

        # ---- u = -2x and its bf16 splits; sx splits
        U = work.tile([P, NT, 3], F32)
        nc.scalar.mul(out=U, in_=Xn, mul=-2.0)
        ub = _split3(nc, work, U, [P, NT, 3], "u")
        sxb = _split3(nc, work, sx, [P, NT], "sx")

        # ---- build stacked X-side aug tile [128, 64, NT] (bf16), then
        # transpose to XS [64, n] (matmul base partitions must be 0/32):
        #   rows 0-23  = dir-1 lhsT: u groups L_LEVELS, sx splits, ones
        #   rows 32-55 = dir-2 rhs : u groups R_LEVELS, ones, sx splits
        XSa = work.tile([P, 64, NT], BF16)
        ASM(nc).memset(XSa[:, 24:32, :], 0.0)
        ASM(nc).memset(XSa[:, 56:64, :], 0.0)
        for g, lv in enumerate(L_LEVELS):
            ASM(nc).tensor_copy(out=XSa[:, 3 * g:3 * g + 3, :],
                                  in_=ub[lv].rearrange("p t c -> p c t"))
        for l in range(3):
            ASM(nc).tensor_copy(out=XSa[:, 18 + l, :], in_=sxb[l])
        ASM(nc).memset(XSa[:, 21:24, :], 1.0)
        for g, lv in enumerate(R_LEVELS):
            ASM(nc).tensor_copy(out=XSa[:, 32 + 3 * g:32 + 3 * g + 3, :],
                                  in_=ub[lv].rearrange("p t c -> p c t"))
        ASM(nc).memset(XSa[:, 50:53, :], 1.0)
        for l in range(3):
            ASM(nc).tensor_copy(out=XSa[:, 53 + l, :], in_=sxb[l])

        # per-chunk tiles so matmuls only wait on the chunk they read
        XS = []
        for g in range(NST):
            pt = pstripe.tile([64, W], BF16, tag="stripe")
            for k in range(W // P):
                t = g * (W // P) + k
                nc.tensor.transpose(pt[:, k * P:(k + 1) * P], XSa[:, :, t], id128)
            xc = const.tile([64, W], BF16, tag=f"XSc{g}")
            nc.scalar.copy(out=xc, in_=pt)
            XS.append(xc)

        # ---- collected mins: [128, 2*NT*H] (dir1 + dir2 per head)
        mins_all = const.tile([P, 2 * NT * H], F32)
        # normalized normals per head (redundant across partitions)
        nhat = const.tile([P, H, 3], F32)

        for h in range(H):
            # --- normalize head normal (per-partition redundant, exact DVE ops)
            sqn = headp.tile([P, 3], F32, tag="sqn")
            nc.vector.tensor_tensor(out=sqn, in0=ypb[:, h, 0:3], in1=ypb[:, h, 0:3],
                                    op=OP.mult)
            nn = headp.tile([P, 1], F32, tag="nn")
            nc.vector.tensor_reduce(out=nn, in_=sqn, axis=AX.X, op=OP.add)
            sq_ = headp.tile([P, 1], F32, tag="sq_")
            nc.scalar.activation(out=sq_, in_=nn, func=AF.Sqrt)
            rs0 = headp.tile([P, 1], F32, tag="rs0")
            nc.vector.reciprocal(out=rs0, in_=sq_)
            # one Newton step: rs = rs0*(1.5 - 0.5*nn*rs0^2)
            a = headp.tile([P, 1], F32, tag="nta")
            nc.vector.tensor_tensor(out=a, in0=rs0, in1=rs0, op=OP.mult)
            nc.vector.tensor_tensor(out=a, in0=a, in1=nn, op=OP.mult)
            nc.vector.tensor_scalar(out=a, in0=a, scalar1=-0.5, scalar2=1.5,
                                    op0=OP.mult, op1=OP.add)
            rs = headp.tile([P, 1], F32, tag="rs")
            nc.vector.tensor_tensor(out=rs, in0=rs0, in1=a, op=OP.mult)
            nc.vector.tensor_scalar(out=nhat[:, h, :], in0=ypb[:, h, 0:3], scalar1=rs,
                                    scalar2=None, op0=OP.mult)
            off = ypb[:, h, 3:4]

            # --- s[p,t] = nhat . x + off   (signed plane distance)
            s = headp.tile([P, NT], F32, tag="s")
            t0 = headp.tile([P, NT], F32, tag="t0")
            nc.vector.tensor_scalar(out=s, in0=Xn[:, :, 0], scalar1=nhat[:, h, 0:1],
                                    scalar2=off, op0=OP.mult, op1=OP.add)
            nc.vector.tensor_scalar(out=t0, in0=Xn[:, :, 1], scalar1=nhat[:, h, 1:2],
                                    scalar2=None, op0=OP.mult)
            nc.vector.tensor_tensor(out=s, in0=s, in1=t0, op=OP.add)
            nc.vector.tensor_scalar(out=t0, in0=Xn[:, :, 2], scalar1=nhat[:, h, 2:3],
                                    scalar2=None, op0=OP.mult)
            nc.vector.tensor_tensor(out=s, in0=s, in1=t0, op=OP.add)

            # --- reflected points Yn = x - 2 s nhat ; sy = sx + 4*off*s
            m2 = headp.tile([P, 3], F32, tag="m2")
            nc.vector.tensor_scalar(out=m2, in0=nhat[:, h, :], scalar1=-2.0,
                                    scalar2=None, op0=OP.mult)
            Yn = headp.tile([P, NT, 3], F32, tag="Yn")
            tc_ = headp.tile([P, NT], F32, tag="tc_")
            for c in range(3):
                nc.vector.tensor_scalar(out=tc_, in0=s, scalar1=m2[:, c:c + 1],
                                        scalar2=None, op0=OP.mult)
                nc.vector.tensor_tensor(out=Yn[:, :, c], in0=Xn[:, :, c], in1=tc_,
                                        op=OP.add)
            o4 = headp.tile([P, 1], F32, tag="o4")
            nc.vector.tensor_scalar(out=o4, in0=off, scalar1=4.0, scalar2=None,
                                    op0=OP.mult)
            sy = headp.tile([P, NT], F32, tag="sy")
            nc.vector.tensor_scalar(out=sy, in0=s, scalar1=o4, scalar2=None,
                                    op0=OP.mult)
            nc.vector.tensor_tensor(out=sy, in0=sy, in1=sx, op=OP.add)

            # --- y / sy splits and stacked Y-side aug tile
            yb = _split3(nc, headp, Yn, [P, NT, 3], "y")
            syb = _split3(nc, headp, sy, [P, NT], "sy")
            YSa = headp.tile([P, 64, NT], BF16, tag="YSa")
            ASM(nc).memset(YSa[:, 24:32, :], 0.0)
            ASM(nc).memset(YSa[:, 56:64, :], 0.0)
            # rows 0-23 = dir-1 rhs: y groups R_LEVELS, ones, sy splits
            for g, lv in enumerate(R_LEVELS):
                ASM(nc).tensor_copy(out=YSa[:, 3 * g:3 * g + 3, :],
                                      in_=yb[lv].rearrange("p t c -> p c t"))
            ASM(nc).memset(YSa[:, 18:21, :], 1.0)
            for l in range(3):
                ASM(nc).tensor_copy(out=YSa[:, 21 + l, :], in_=syb[l])
            # rows 32-55 = dir-2 lhsT: y groups L_LEVELS, sy splits, ones
            for g, lv in enumerate(L_LEVELS):
                ASM(nc).tensor_copy(out=YSa[:, 32 + 3 * g:32 + 3 * g + 3, :],
                                      in_=yb[lv].rearrange("p t c -> p c t"))
            for l in range(3):
                ASM(nc).tensor_copy(out=YSa[:, 50 + l, :], in_=syb[l])
            ASM(nc).memset(YSa[:, 53:56, :], 1.0)

            YS = []
            for g in range(NST):
                pt = pstripe.tile([64, W], BF16, tag="stripe")
                for k in range(W // P):
                    t = g * (W // P) + k
                    nc.tensor.transpose(pt[:, k * P:(k + 1) * P], YSa[:, :, t], id128)
                yc = headp.tile([64, W], BF16, tag=f"YSc{g}")
                nc.scalar.copy(out=yc, in_=pt)
                YS.append(yc)

            # --- main loops, both directions interleaved per 128-row
            # block. Two block flavors:
            #  fp16-tree: ACT evacuates all stripes PSUM->SBUF fp16; DVE
            #    does a 2x-mode fp16 TT-min tree + one fp16 reduce_min.
            #  fp32-chain: DVE TT-mins PSUM stripes into a running SBUF min
            #    (no ACT work) + one fp32 reduce_min.
            def emit_block_f16(d2, i):
                if d2 == 0:
                    LT, RT, lo = XS, YS, 0
                else:
                    LT, RT, lo = YS, XS, 32
                lc, lof = (i * P) // W, (i * P) % W
                lhsT = LT[lc][lo:lo + 24, lof:lof + P]
                rowcol = mins_all[:, h * 2 * NT + d2 * NT + i:
                                  h * 2 * NT + d2 * NT + i + 1]
                sb = []
                for g in range(NST):
                    ps = pstripe.tile([P, W], F32, tag="stripe")
                    for m in range(NMM):
                        nc.tensor.matmul(
                            ps[:, m * 512:(m + 1) * 512],
                            lhsT=lhsT,
                            rhs=RT[g][lo:lo + 24,
                                      m * 512:(m + 1) * 512],
                            start=True, stop=True)
                    s16 = work.tile([P, W], FP16, tag=f"f16s{g % 4}")
                    nc.scalar.copy(out=s16, in_=ps)
                    sb.append(s16)
                m0 = work.tile([P, W], FP16, tag="f16m0")
                nc.vector.tensor_tensor(out=m0, in0=sb[0], in1=sb[1], op=OP.min)
                if NST >= 4:
                    m1 = work.tile([P, W], FP16, tag="f16m1")
                    nc.vector.tensor_tensor(out=m1, in0=sb[2], in1=sb[3],
                                            op=OP.min)
                    nc.vector.tensor_tensor(out=m0, in0=m0, in1=m1, op=OP.min)
                # narrow with 2x-mode fp16 TT halvings before the 1x reduce
                n1 = work.tile([P, W // 2], FP16, tag="f16n1")
                nc.vector.tensor_tensor(out=n1, in0=m0[:, 0:W // 2],
                                        in1=m0[:, W // 2:W], op=OP.min)
                n2 = work.tile([P, W // 4], FP16, tag="f16n2")
                nc.vector.tensor_tensor(out=n2, in0=n1[:, 0:W // 4],
                                        in1=n1[:, W // 4:W // 2], op=OP.min)
                nc.vector.tensor_reduce(out=rowcol, in_=n2, axis=AX.X,
                                        op=OP.min)

            def emit_block_f32chain(d2, i):
                if d2 == 0:
                    LT, RT, lo = XS, YS, 0
                else:
                    LT, RT, lo = YS, XS, 32
                lc, lof = (i * P) // W, (i * P) % W
                lhsT = LT[lc][lo:lo + 24, lof:lof + P]
                rowcol = mins_all[:, h * 2 * NT + d2 * NT + i:
                                  h * 2 * NT + d2 * NT + i + 1]
                mm = work.tile([P, W], F32, tag="chainmm")
                mm16 = work.tile([P, W], FP16, tag="chainmm16")
                for g in range(NST):
                    ps = pstripe.tile([P, W], F32, tag="stripe")
                    for m in range(NMM):
                        nc.tensor.matmul(
                            ps[:, m * 512:(m + 1) * 512],
                            lhsT=lhsT,
                            rhs=RT[g][lo:lo + 24,
                                      m * 512:(m + 1) * 512],
                            start=True, stop=True)
                    if g == 0:
                        nc.scalar.copy(out=mm, in_=ps)
                    else:
                        # last level writes fp16 so the tail can use
                        # 2x-mode fp16 TT narrowing instead of a 1x reduce
                        dst = mm16 if g == NST - 1 else mm
                        nc.vector.tensor_tensor(out=dst, in0=ps, in1=mm,
                                                op=OP.min)
                if NST > 1:
                    c1 = work.tile([P, W // 2], FP16, tag="chainn1")
                    nc.vector.tensor_tensor(out=c1, in0=mm16[:, 0:W // 2],
                                            in1=mm16[:, W // 2:W], op=OP.min)
                    c2 = work.tile([P, W // 4], FP16, tag="chainn2")
                    nc.vector.tensor_tensor(out=c2, in0=c1[:, 0:W // 4],
                                            in1=c1[:, W // 4:W // 2], op=OP.min)
                    nc.vector.tensor_reduce(out=rowcol, in_=c2, axis=AX.X,
                                            op=OP.min)
                else:
                    nc.vector.tensor_reduce(out=rowcol, in_=mm, axis=AX.X,
                                            op=OP.min)

            def emit_block_unpaired(d2, i):
                if d2 == 0:
                    LT, RT, lo = XS, YS, 0
                else:
                    LT, RT, lo = YS, XS, 32
                lc, lof = (i * P) // W, (i * P) % W
                lhsT = LT[lc][lo:lo + 24, lof:lof + P]
                rowcol = mins_all[:, h * 2 * NT + d2 * NT + i:
                                  h * 2 * NT + d2 * NT + i + 1]
                ps = pstripe.tile([P, W], F32, tag="stripe")
                for m in range(NMM):
                    nc.tensor.matmul(
                        ps[:, m * 512:(m + 1) * 512],
                        lhsT=lhsT,
                        rhs=RT[0][lo:lo + 24, m * 512:(m + 1) * 512],
                        start=True, stop=True)
                nc.vector.tensor_reduce(out=rowcol, in_=ps, axis=AX.X,
                                        op=OP.min)

            # fp32-chain on 3/7 of blocks balances ACT vs DVE
            for i in range(NT):
                for d2 in range(2):
                    if NST == 1:
                        emit_block_unpaired(d2, i)
                    elif ((2 * i + d2) * 3) % 7 < 3:
                        emit_block_f32chain(d2, i)
                    else:
                        emit_block_f16(d2, i)

        # ---- regularizer: reg = sqrt(sum((Nhat Nhat^T - I)^2)), computed
        # redundantly across partitions with exact DVE ops.
        gsq = work.tile([P, 9], F32, tag="gsq")
        gtmp = work.tile([P, 3], F32, tag="gtmp")
        for m in range(3):
            for nn_ in range(3):
                nc.vector.tensor_tensor(out=gtmp, in0=nhat[:, m, :], in1=nhat[:, nn_, :],
                                        op=OP.mult)
                g1 = gsq[:, 3 * m + nn_:3 * m + nn_ + 1]
                nc.vector.tensor_reduce(out=g1, in_=gtmp, axis=AX.X, op=OP.add)
                if m == nn_:
                    nc.vector.tensor_scalar(out=g1, in0=g1, scalar1=-1.0,
                                            scalar2=None, op0=OP.add)
        nc.vector.tensor_tensor(out=gsq, in0=gsq, in1=gsq, op=OP.mult)
        q = work.tile([P, 1], F32, tag="q")
        nc.vector.tensor_reduce(out=q, in_=gsq, axis=AX.X, op=OP.add)
        sq0 = work.tile([P, 1], F32, tag="sq0")
        nc.scalar.activation(out=sq0, in_=q, func=AF.Sqrt)
        # Newton polish: sqrt = 0.5*(sq0 + q/sq0)
        rcp = work.tile([P, 1], F32, tag="rcp")
        nc.vector.reciprocal(out=rcp, in_=sq0)
        nc.vector.tensor_tensor(out=rcp, in0=rcp, in1=q, op=OP.mult)
        nc.vector.tensor_tensor(out=rcp, in0=rcp, in1=sq0, op=OP.add)
        reg = work.tile([P, 1], F32, tag="reg")
        nc.vector.tensor_scalar(out=reg, in0=rcp, scalar1=0.5 * REG_COEF,
                                scalar2=None, op0=OP.mult)

        # ---- final: sum(mins_all) over free dim, fold partitions, add reg
        sv = work.tile([P, 1], F32, tag="sv")
        nc.vector.tensor_reduce(out=sv, in_=mins_all, axis=AX.X, op=OP.add)
        # partition column -> single-partition row (exact, via DMA), then reduce
        row = work.tile([1, P], F32, tag="foldrow")
        nc.sync.dma_start(out=row, in_=sv)
        tot = work.tile([1, 1], F32, tag="tot")
        nc.vector.tensor_reduce(out=tot, in_=row, axis=AX.X, op=OP.add)
        final = work.tile([1, 1], F32, tag="final")
        nc.vector.tensor_tensor(out=final, in0=tot, in1=reg[0:1, :], op=OP.add)
        nc.sync.dma_start(out=out, in_=final)


_CACHE = {}


def _get_nc(n=4096):
    if n not in _CACHE:
        nc = bacc.Bacc("TRN2", target_bir_lowering=False, debug=False,
                       num_devices=B)
        emit_chamfer(nc, n)
        nc.compile()
        _CACHE[n] = nc
    return _CACHE[n]


def kernel(sample_points: np.ndarray, y_pred: np.ndarray) -> np.ndarray:
    assert sample_points.shape == (B, 4096, 3)
    assert y_pred.shape == (B, H, 4)
    nc = _get_nc(4096)
    in_maps = [
        {"pts": np.ascontiguousarray(sample_points[b], dtype=np.float32),
         "yp": np.ascontiguousarray(y_pred[b], dtype=np.float32)}
        for b in range(B)
    ]
    # the axon-tunneled device pool occasionally reports a transiently
    # wedged core; retry a few times before giving up
    import time as _time
    last_err = None
    for attempt in range(4):
        try:
            res = run_bass_kernel_spmd(nc, in_maps, list(range(B)))
            break
        except Exception as e:  # noqa: BLE001
            last_err = e
            _time.sleep(3.0 * (attempt + 1))
    else:
        raise last_err
    total = np.float64(0.0)
    for b in range(B):
        total += np.float64(res.results[b]["out"][0, 0])
    return np.asarray(total, dtype=np.float32).reshape(())



# revision 11
# speedup vs baseline: 2.0664x; 2.0664x over previous
"""Trainium2 Bass kernel for nn_ChamferLoss (reflection-symmetry chamfer loss).

Sharding: pure data parallel - batch b -> core b (B=8, 8 cores). Each core
computes its batch's loss; the host sums the 8 scalar partials.

Key identity: reflection R is an involutive isometry, so
|x_i - R x_j| = |R x_i - x_j|, hence both chamfer directions are the SAME
sum: chamfer_bidir(X, R X) = 2 * sum_i min_j |R x_i - x_j|^2. Only ONE
distance matrix per head (3 instead of the naive 6) and one min-reduction
pass per element are needed.

Distance matmul computes NEGATED distances -d = 2y.x - |y|^2 - |x|^2 so all
reductions are maxes (pool/DVE-friendly). fp32 operands are decomposed into
2 bf16 levels (hh, hm, mh cross terms; ~2^-16 relative accuracy, far inside
the 2e-2 gate) stacked along K=16 -> full-speed bf16 matmuls. The moving
(rhs) side is the x cloud, shared by all heads; the stationary (lhsT) side
is the reflected y block, rebuilt per head via PE transposes.

Per 128-row block the [128, 4096] PSUM row is max-reduced by a per-block
plan chosen greedily to balance the ACT/DVE/Pool engines (GPSIMD cannot
touch PSUM, so Pool only works on evacuated fp16 SBUF tiles): each stripe
pair is consumed either by a DVE TT-max straight from PSUM or by an ACT
evacuation (f32->fp16) followed by a DVE/Pool fp16 TT-max; the remaining
fp16 tree stages (1024/512/256 + final DVE reduce) are split between DVE
(2x mode) and Pool.
"""

import sys

sys.path.insert(0, "/opt/trn_rl_repo")

from contextlib import ExitStack

import numpy as np

import concourse.bass as bass
import concourse.bacc as bacc
import concourse.tile as tile
from concourse import mybir
from concourse.masks import make_identity
from concourse.bass_utils import run_bass_kernel_spmd

F32 = mybir.dt.float32
BF16 = mybir.dt.bfloat16
FP16 = mybir.dt.float16
AX = mybir.AxisListType
OP = mybir.AluOpType
AF = mybir.ActivationFunctionType

P = 128
H = 3
REG_COEF = 25.0
B = 8

# cost-model per-op engine times (ns) used by the greedy pair balancer
C_ACT_EVAC = 1297.7   # ACT copy [128,1024] psum f32 -> sbuf f16
C_DVE_PAIR = 1411.3   # DVE TT-max [128,1024] psum f32 x sbuf f16 -> f16
C_DVE_COPY = 1392.6   # DVE tensor_copy [128,1024] psum f32 -> sbuf f16
C_DVE_1024 = 651.8    # DVE TT-max f16 sbuf [128,1024] (2x mode)
C_DVE_TAIL = 1314.3   # per-block share of the 4-block-batched tail tree
NB = 4                # tail batch size (blocks per batched tree)


def _plan_blocks(total, init=(0.0, 0.0)):
    """Greedy per-block stripe-pair assignment minimizing the running max
    of (ACT, DVE) busy time.

    Per block: two PSUM stripe-pairs each consumed by one of
      PDD: DVE copy of stripe A + DVE TT-max(stripe B psum, A)
      EDT: ACT evac of stripe A + DVE TT-max(stripe B psum, A)
      ED:  ACT evac both stripes + DVE f16 TT
    (hardware: TT reads at most one PSUM input; GPSIMD has no max; DMA
    accumulate has no max -- so only ACT and DVE can consume distances.)
    The per-block tail trees are batched NB blocks at a time on DVE.
    Returns list of (pair0, pair1).
    """
    pair_opts = {
        "PDD": (0.0, C_DVE_COPY + C_DVE_PAIR),
        "EDT": (C_ACT_EVAC, C_DVE_PAIR),
        "ED": (2 * C_ACT_EVAC, C_DVE_1024),
    }
    load = list(init)
    plans = []
    for _ in range(total):
        best = None
        for p0 in pair_opts:
            for p1 in pair_opts:
                cost = [load[0], load[1] + C_DVE_TAIL]
                for j in range(2):
                    cost[j] += pair_opts[p0][j] + pair_opts[p1][j]
                key = (max(cost), sum(cost))
                if best is None or key < best[0]:
                    best = (key, (p0, p1), cost)
        plans.append(best[1])
        load = best[2]
    return plans


def _split2(nc, pool, src, shape, tag):
    """2-level bf16 split of an f32 tile: src ~= b0+b1 (rel ~2^-16)."""
    b0 = pool.tile(shape, BF16, tag=f"{tag}b0")
    nc.scalar.copy(out=b0, in_=src)
    r = pool.tile(shape, F32, tag=f"{tag}r")
    nc.vector.tensor_tensor(out=r, in0=src, in1=b0, op=OP.subtract)
    b1 = pool.tile(shape, BF16, tag=f"{tag}b1")
    nc.scalar.copy(out=b1, in_=r)
    return b0, b1


def emit_chamfer(nc, n=4096):
    NT = n // P           # number of 128-point blocks (32)
    W = 1024              # psum stripe width (one matmul each)
    NST = n // W          # stripes per row-block (4)

    pts = nc.dram_tensor("pts", [n, 3], F32, kind="ExternalInput").ap()
    yp = nc.dram_tensor("yp", [H, 4], F32, kind="ExternalInput").ap()
    out = nc.dram_tensor("out", [1, 1], F32, kind="ExternalOutput").ap()

    plans = _plan_blocks(H * NT)

    with ExitStack() as ctx:
        tc = ctx.enter_context(tile.TileContext(nc))
        const = ctx.enter_context(tc.tile_pool(name="const", bufs=1))
        work = ctx.enter_context(tc.tile_pool(name="work", bufs=4))
        headp = ctx.enter_context(tc.tile_pool(name="headp", bufs=2))
        pstripe = ctx.enter_context(tc.tile_pool(
            name="pstripe", bufs=4, space="PSUM"))

        id128 = const.tile([P, P], BF16)
        make_identity(nc, id128)

        # ---- load points: Xn[p, t, c] = pts[t*128+p, c]
        Xn = const.tile([P, NT, 3], F32)
        nc.sync.dma_start(out=Xn, in_=pts.rearrange("(t p) c -> p t c", p=P))

        # ---- yp broadcast to all partitions: ypb[p, h, k] = yp[h, k]
        ypb = const.tile([P, H, 4], F32)
        yp_b = bass.AP(tensor=yp.tensor, offset=yp.offset,
                       ap=[[0, P], [4, H], [1, 4]])
        nc.sync.dma_start(out=ypb, in_=yp_b)

        # ---- sx = |x|^2 per point, in [128, NT] layout
        Xsq = work.tile([P, NT, 3], F32)
        nc.scalar.activation(out=Xsq, in_=Xn, func=AF.Square)
        sx = const.tile([P, NT], F32)
        nc.vector.tensor_tensor(out=sx, in0=Xsq[:, :, 0], in1=Xsq[:, :, 1],
                                op=OP.add)
        nc.vector.tensor_tensor(out=sx, in0=sx, in1=Xsq[:, :, 2], op=OP.add)

        # ---- x bf16 2-level splits and -sx splits
        xh, xm = _split2(nc, work, Xn, [P, NT, 3], "x")
        nsx = work.tile([P, NT], F32, tag="nsx")
        nc.vector.tensor_scalar(out=nsx, in0=sx, scalar1=-1.0, scalar2=None,
                                op0=OP.mult)
        nsxb0, nsxb1 = _split2(nc, work, nsx, [P, NT], "nsx")

        # ---- stacked x-side tile SX[p, t, r] (bf16), r in [0,16):
        #   0-2 x_h | 3-5 x_m | 6-8 x_h | 9,10 ones | 11 -sx_b0 | 12 -sx_b1
        SX = const.tile([P, NT, 16], BF16)
        nc.gpsimd.memset(SX[:, :, 13:16], 0.0)
        nc.gpsimd.tensor_copy(out=SX[:, :, 0:3], in_=xh)
        nc.gpsimd.tensor_copy(out=SX[:, :, 3:6], in_=xm)
        nc.gpsimd.tensor_copy(out=SX[:, :, 6:9], in_=xh)
        nc.gpsimd.memset(SX[:, :, 9:11], 1.0)
        nc.gpsimd.tensor_copy(out=SX[:, :, 11], in_=nsxb0)
        nc.gpsimd.tensor_copy(out=SX[:, :, 12], in_=nsxb1)

        # ---- moving operand RX[r, j] = SX[j%128, j//128, r] via PE
        # transposes, in two [16, 2048] psum halves. The 16 rows are then
        # replicated to base partitions 32/64/96 (matmul requires lhsT and
        # rhs to share a base partition, and lhsT cycles through the four
        # 32-row quadrants).
        RX = const.tile([P, n], BF16)
        for half in range(2):
            rxp = pstripe.tile([16, 2048], BF16, tag="stripe")
            for tl in range(16):
                t = half * 16 + tl
                nc.tensor.transpose(rxp[:, tl * P:(tl + 1) * P],
                                    SX[:, t, :], id128)
            nc.scalar.copy(out=RX[0:16, half * 2048:(half + 1) * 2048],
                           in_=rxp)
        for quad in range(1, 3):
            nc.sync.dma_start(out=RX[32 * quad:32 * quad + 16, :],
                              in_=RX[0:16, :])

        # ---- collected row maxes of -d: [128, H*NT]
        mins_all = const.tile([P, H * NT], F32)
        # normalized normals per head (redundant across partitions)
        nhat = const.tile([P, H, 3], F32)

        for h in range(H):
            # --- normalize head normal (per-partition redundant, exact ops)
            sqn = headp.tile([P, 3], F32, tag="sqn")
            nc.vector.tensor_tensor(out=sqn, in0=ypb[:, h, 0:3],
                                    in1=ypb[:, h, 0:3], op=OP.mult)
            nn = headp.tile([P, 1], F32, tag="nn")
            nc.vector.tensor_reduce(out=nn, in_=sqn, axis=AX.X, op=OP.add)
            sq_ = headp.tile([P, 1], F32, tag="sq_")
            nc.scalar.activation(out=sq_, in_=nn, func=AF.Sqrt)
            rs0 = headp.tile([P, 1], F32, tag="rs0")
            nc.vector.reciprocal(out=rs0, in_=sq_)
            # one Newton step: rs = rs0*(1.5 - 0.5*nn*rs0^2)
            a = headp.tile([P, 1], F32, tag="nta")
            nc.vector.tensor_tensor(out=a, in0=rs0, in1=rs0, op=OP.mult)
            nc.vector.tensor_tensor(out=a, in0=a, in1=nn, op=OP.mult)
            nc.vector.tensor_scalar(out=a, in0=a, scalar1=-0.5, scalar2=1.5,
                                    op0=OP.mult, op1=OP.add)
            rs = headp.tile([P, 1], F32, tag="rs")
            nc.vector.tensor_tensor(out=rs, in0=rs0, in1=a, op=OP.mult)
            nc.vector.tensor_scalar(out=nhat[:, h, :], in0=ypb[:, h, 0:3],
                                    scalar1=rs, scalar2=None, op0=OP.mult)
            off = ypb[:, h, 3:4]

            # --- s[p,t] = nhat . x + off   (signed plane distance)
            s = headp.tile([P, NT], F32, tag="s")
            t0 = headp.tile([P, NT], F32, tag="t0")
            nc.vector.tensor_scalar(out=s, in0=Xn[:, :, 0],
                                    scalar1=nhat[:, h, 0:1], scalar2=off,
                                    op0=OP.mult, op1=OP.add)
            nc.vector.tensor_scalar(out=t0, in0=Xn[:, :, 1],
                                    scalar1=nhat[:, h, 1:2], scalar2=None,
                                    op0=OP.mult)
            nc.vector.tensor_tensor(out=s, in0=s, in1=t0, op=OP.add)
            nc.vector.tensor_scalar(out=t0, in0=Xn[:, :, 2],
                                    scalar1=nhat[:, h, 2:3], scalar2=None,
                                    op0=OP.mult)
            nc.vector.tensor_tensor(out=s, in0=s, in1=t0, op=OP.add)

            # --- reflected points scaled by 2: Y2 = 2x - 4 s nhat
            m4 = headp.tile([P, 3], F32, tag="m4")
            nc.vector.tensor_scalar(out=m4, in0=nhat[:, h, :], scalar1=-4.0,
                                    scalar2=None, op0=OP.mult)
            Y2 = headp.tile([P, NT, 3], F32, tag="Y2")
            tc_ = headp.tile([P, NT], F32, tag="tc_")
            for c in range(3):
                nc.vector.tensor_scalar(out=tc_, in0=s,
                                        scalar1=m4[:, c:c + 1],
                                        scalar2=None, op0=OP.mult)
                nc.vector.tensor_scalar(out=Y2[:, :, c], in0=Xn[:, :, c],
                                        scalar1=2.0, scalar2=None,
                                        op0=OP.mult)
                nc.vector.tensor_tensor(out=Y2[:, :, c], in0=Y2[:, :, c],
                                        in1=tc_, op=OP.add)
            # --- -sy = -|y|^2 = -sx - 4*off*s
            no4 = headp.tile([P, 1], F32, tag="no4")
            nc.vector.tensor_scalar(out=no4, in0=off, scalar1=-4.0,
                                    scalar2=None, op0=OP.mult)
            nsy = headp.tile([P, NT], F32, tag="nsy")
            nc.vector.tensor_scalar(out=nsy, in0=s, scalar1=no4,
                                    scalar2=None, op0=OP.mult)
            nc.vector.tensor_tensor(out=nsy, in0=nsy, in1=nsx, op=OP.add)

            # --- y-side bf16 splits and stacked tile STY[p, t, r]:
            #   0-2 2y_h | 3-5 2y_h | 6-8 2y_m | 9 -sy_b0 | 10 -sy_b1
            #   | 11,12 ones | 13-31 zero
            yh, ym = _split2(nc, headp, Y2, [P, NT, 3], "y")
            nsyb0, nsyb1 = _split2(nc, headp, nsy, [P, NT], "nsy")
            STY = headp.tile([P, NT, 32], BF16, tag="STY")
            nc.gpsimd.memset(STY[:, :, 13:32], 0.0)
            nc.gpsimd.tensor_copy(out=STY[:, :, 0:3], in_=yh)
            nc.gpsimd.tensor_copy(out=STY[:, :, 3:6], in_=yh)
            nc.gpsimd.tensor_copy(out=STY[:, :, 6:9], in_=ym)
            nc.gpsimd.tensor_copy(out=STY[:, :, 9], in_=nsyb0)
            nc.gpsimd.tensor_copy(out=STY[:, :, 10], in_=nsyb1)
            nc.gpsimd.memset(STY[:, :, 11:13], 1.0)

            # --- stationary operands: transpose 3 blocks at a time
            # (matmul base partitions are limited to 0/32/64)
            NCH = (NT + 2) // 3
            LH = headp.tile([P, NCH, P], BF16, tag="LH")
            for g in range(NCH):
                nblk = min(3, NT - 3 * g)
                pt = pstripe.tile([P, P], BF16, tag="stripe")
                nc.tensor.transpose(
                    pt[0:32 * nblk, :],
                    STY[:, 3 * g:3 * g + nblk, :].rearrange(
                        "p a b -> p (a b)"), id128)
                nc.scalar.copy(out=LH[0:32 * nblk, g, :],
                               in_=pt[0:32 * nblk, :])

            # --- main block loop: one [128, 4096] row of -d per block
            for i in range(NT):
                bi = h * NT + i
                p0, p1 = plans[bi]
                lhsT = LH[32 * (i % 3):32 * (i % 3) + 16, i // 3, :]
                sb = []
                quad = 32 * (i % 3)
                for g in range(NST):
                    ps = pstripe.tile([P, W], F32, tag="stripe")
                    for mmo in range(0, W, 512):
                        nc.tensor.matmul(
                            ps[:, mmo:mmo + 512], lhsT=lhsT,
                            rhs=RX[quad:quad + 16,
                                   g * W + mmo:g * W + mmo + 512],
                            start=True, stop=True)
                    sb.append(ps)

                if i % NB == 0:
                    mA = work.tile([P, NB, W], FP16, tag="mA")
                    mB = work.tile([P, NB, W], FP16, tag="mB")
                for pi, pk in enumerate((p0, p1)):
                    sa, sc = sb[2 * pi], sb[2 * pi + 1]
                    m = (mA if pi == 0 else mB)[:, i % NB, :]
                    if pk in ("PDD", "EDT"):
                        e = work.tile([P, W], FP16, tag=f"ev{pi}")
                        if pk == "PDD":
                            nc.vector.tensor_copy(out=e, in_=sa)
                        else:
                            nc.scalar.copy(out=e, in_=sa)
                        nc.vector.tensor_tensor(out=m, in0=sc, in1=e,
                                                op=OP.max)
                    else:
                        e = work.tile([P, 2 * W], FP16, tag=f"ew{pi}")
                        nc.scalar.copy(out=e[:, 0:W], in_=sa)
                        nc.scalar.copy(out=e[:, W:2 * W], in_=sc)
                        nc.vector.tensor_tensor(out=m, in0=e[:, 0:W],
                                                in1=e[:, W:2 * W], op=OP.max)

                if i % NB == NB - 1:
                    # batched tail tree over the last NB blocks
                    t1 = work.tile([P, NB, W], FP16, tag="t1")
                    nc.vector.tensor_tensor(out=t1, in0=mA, in1=mB,
                                            op=OP.max)
                    t2 = work.tile([P, NB, W // 2], FP16, tag="t2")
                    nc.vector.tensor_tensor(out=t2, in0=t1[:, :, 0:W // 2],
                                            in1=t1[:, :, W // 2:W],
                                            op=OP.max)
                    t3 = work.tile([P, NB, W // 4], FP16, tag="t3")
                    nc.vector.tensor_tensor(out=t3, in0=t2[:, :, 0:W // 4],
                                            in1=t2[:, :, W // 4:W // 2],
                                            op=OP.max)
                    nc.vector.tensor_reduce(
                        out=mins_all[:, bi - NB + 1:bi + 1], in_=t3,
                        axis=AX.X, op=OP.max)

        # ---- regularizer: reg = sqrt(sum((Nhat Nhat^T - I)^2)), computed
        # redundantly across partitions with exact DVE ops.
        gsq = work.tile([P, 9], F32, tag="gsq")
        gtmp = work.tile([P, 3], F32, tag="gtmp")
        for m in range(3):
            for nn_ in range(3):
                nc.vector.tensor_tensor(out=gtmp, in0=nhat[:, m, :],
                                        in1=nhat[:, nn_, :], op=OP.mult)
                g1 = gsq[:, 3 * m + nn_:3 * m + nn_ + 1]
                nc.vector.tensor_reduce(out=g1, in_=gtmp, axis=AX.X,
                                        op=OP.add)
                if m == nn_:
                    nc.vector.tensor_scalar(out=g1, in0=g1, scalar1=-1.0,
                                            scalar2=None, op0=OP.add)
        nc.vector.tensor_tensor(out=gsq, in0=gsq, in1=gsq, op=OP.mult)
        q = work.tile([P, 1], F32, tag="q")
        nc.vector.tensor_reduce(out=q, in_=gsq, axis=AX.X, op=OP.add)
        sq0 = work.tile([P, 1], F32, tag="sq0")
        nc.scalar.activation(out=sq0, in_=q, func=AF.Sqrt)
        # Newton polish: sqrt = 0.5*(sq0 + q/sq0)
        rcp = work.tile([P, 1], F32, tag="rcp")
        nc.vector.reciprocal(out=rcp, in_=sq0)
        nc.vector.tensor_tensor(out=rcp, in0=rcp, in1=q, op=OP.mult)
        nc.vector.tensor_tensor(out=rcp, in0=rcp, in1=sq0, op=OP.add)
        reg = work.tile([P, 1], F32, tag="reg")
        nc.vector.tensor_scalar(out=reg, in0=rcp, scalar1=0.5 * REG_COEF,
                                scalar2=None, op0=OP.mult)

        # ---- final: chamfer = -2 * sum(maxes of -d); add reg
        sv = work.tile([P, 1], F32, tag="sv")
        nc.vector.tensor_reduce(out=sv, in_=mins_all, axis=AX.X, op=OP.add)
        # partition column -> single-partition row (exact, via DMA)
        row = work.tile([1, P], F32, tag="foldrow")
        nc.sync.dma_start(out=row, in_=sv)
        tot = work.tile([1, 1], F32, tag="tot")
        nc.vector.tensor_reduce(out=tot, in_=row, axis=AX.X, op=OP.add)
        nc.vector.tensor_scalar(out=tot, in0=tot, scalar1=-2.0, scalar2=None,
                                op0=OP.mult)
        final = work.tile([1, 1], F32, tag="final")
        nc.vector.tensor_tensor(out=final, in0=tot, in1=reg[0:1, :],
                                op=OP.add)
        nc.sync.dma_start(out=out, in_=final)


_CACHE = {}


def _get_nc(n=4096):
    if n not in _CACHE:
        nc = bacc.Bacc("TRN2", target_bir_lowering=False, debug=False,
                       num_devices=B)
        emit_chamfer(nc, n)
        nc.compile()
        _CACHE[n] = nc
    return _CACHE[n]


def kernel(sample_points: np.ndarray, y_pred: np.ndarray) -> np.ndarray:
    assert sample_points.shape == (B, 4096, 3)
    assert y_pred.shape == (B, H, 4)
    nc = _get_nc(4096)
    in_maps = [
        {"pts": np.ascontiguousarray(sample_points[b], dtype=np.float32),
         "yp": np.ascontiguousarray(y_pred[b], dtype=np.float32)}
        for b in range(B)
    ]
    # the axon-tunneled device pool occasionally reports a transiently
    # wedged core; retry a few times before giving up
    import time as _time
    last_err = None
    for attempt in range(4):
        try:
            res = run_bass_kernel_spmd(nc, in_maps, list(range(B)))
            break
        except Exception as e:  # noqa: BLE001
            last_err = e
            _time.sleep(3.0 * (attempt + 1))
    else:
        raise last_err
    total = np.float64(0.0)
    for b in range(B):
        total += np.float64(res.results[b]["out"][0, 0])
    return np.asarray(total, dtype=np.float32).reshape(())
